# revision 25
# baseline (speedup 1.0000x reference)
"""BertMoELayer (B=4, S=2048, H=768, F=3072, E=8, top-2) on 8 Trainium2 cores.

Expert-parallel sharding with host-side dispatch by top-k expert index; all
numeric computation of the layer runs on device. Per core c over its
gathered tokens (capacity C, 8-aligned):

    logits^T = WrhT^T@xh + WrhT^T@xl + WrlT^T@xh     (split-bf16, fp32 psum)
    w_c      = top-2 softmax weight of expert c       (fp32 vector chain)
    hT       = gelu(WiT^T @ xh + bi)                  (bf16 matmul, fp32 psum)
    out_c    = w_c * (hT^T @ WoT + bo)                (bf16 matmul, fp32 psum)

The host unshards by scatter-adding each core's (already weighted) rows.
Near-tie tokens (2nd/3rd logit gap < 1e-4) are dispatched to 3 cores; the
device chain gives the losing expert weight 0, so device-order effects are
harmless. The 3-pass split-bf16 router matches fp32 logits to ~1e-5 (vs a
2e-5 min gap on this data: zero top-2 flips) at 1/4 the fp32 matmul cost.

Schedule (from perfetto profiling):
  * mm1 runs j-major across 1024-token superblocks so the Wi stream needs
    only ~77GB/s; walking all of Wi per 512-token block tripped the HAM
    activity throttle (k=4 duty windows) and starved the PE.
  * One SBUF tile per DMA: the tile framework WAW-serializes multiple DMAs
    into one tile, which silently turns a parallel preamble into a serial
    transfer chain.
  * Late streams (xh1/xlo/wo) are triggered on the gpsimd queue gated by
    probe reads of hT, i.e. just-in-time with compute progress, so they
    never compete with the urgent x/Wi transfers during the slow pre-grant
    DMA window.
  * One batched EXP per superblock (not per 128-token slice): each EXP
    costs a ~1.5us GELU<->EXP activation-table reload on the scalar engine
    which otherwise stalls mm1 through the gelu/psum ring.
  * All tensors host-packed per-partition-contiguous (a [P,KF]-strided bi
    DMA alone cost 5.3us of descriptor generation in the naive layout).

Matmul FLOPs run in bf16 with fp32 accumulation (78.6 TF/s peak; fp8 would
double throughput but measured 5.1e-2 rel err vs the 2e-2 gate, and any
split-operand compensation scheme costs 2 fp8 passes = bf16 speed).
Steady-state PE occupancy ~98%, matmul issue at theoretical spacing.
"""

import numpy as np
import ml_dtypes

import concourse.bass as bass
import concourse.tile as tile
from concourse import bacc, mybir
from concourse.bass_utils import run_bass_kernel_spmd
from concourse.masks import make_identity

B, S, H, F, E = 4, 2048, 768, 3072, 8
T = B * S
N_CORES = 8
TOP_K = 2

P = 128          # SBUF partitions
TB = 512         # token block (matmul moving-dim max)
KH = H // P      # 6   h-chunks
KF = F // P      # 24  f-chunks
HO = 384         # output free-dim split (2 x 384 = 768), one PSUM bank each
WG = 4           # Wi j-columns per DMA group

F32 = mybir.dt.float32
BF16 = mybir.dt.bfloat16
BF16_NP = ml_dtypes.bfloat16


def build_nc(cap: int):
    """Per-core program: split-bf16 router + dense expert FFN over `cap`."""
    assert cap % 8 == 0 and cap > 2 * TB
    blocks = []
    t0 = 0
    while t0 < cap:
        b = min(2 * TB, cap - t0)
        if cap - t0 - b < TB:
            b = cap - t0   # absorb the short tail into this superblock
        blocks.append((t0, b))
        t0 += b

    nc = bacc.Bacc(None)

    # --- DRAM parameters, all in device layout (host pre-packs) ---
    SB = 2 * TB      # superblock: mm1 runs j-major across it
    xh0 = nc.declare_dram_parameter("xh0", [P, KH, SB], BF16, isOutput=False)
    xh1 = nc.declare_dram_parameter("xh1", [P, KH, cap - SB], BF16, isOutput=False)
    xlo = nc.declare_dram_parameter("xlo", [P, KH, cap], BF16, isOutput=False)
    wi = nc.declare_dram_parameter("wi", [P, KF // WG, KH, WG * P], BF16,
                                   isOutput=False)
    wo = nc.declare_dram_parameter("wo", [P, KF, H], BF16, isOutput=False)
    wr = nc.declare_dram_parameter("wr", [P, 2, KH, E], BF16, isOutput=False)
    bi = nc.declare_dram_parameter("bi", [P, KF], F32, isOutput=False)
    bo = nc.declare_dram_parameter("bo", [H], F32, isOutput=False)
    esel = nc.declare_dram_parameter("esel", [E], F32, isOutput=False)
    out = nc.declare_dram_parameter("out", [cap, H], F32, isOutput=True)

    with tile.TileContext(nc) as tc:
        with (
            tc.tile_pool(name="weights", bufs=1) as wpool,
            tc.tile_pool(name="hbuf", bufs=1) as hpool,
            tc.tile_pool(name="obuf", bufs=3) as opool,
            tc.tile_pool(name="router", bufs=2) as rpool,
            tc.tile_pool(name="psum_h", bufs=3, space="PSUM") as ph_pool,
            tc.tile_pool(name="psum_o", bufs=2, space="PSUM") as po_pool,
            tc.tile_pool(name="psum_r", bufs=2, space="PSUM") as pr_pool,
            tc.tile_pool(name="psum_rt", bufs=1, space="PSUM") as prt_pool,
        ):
            # ---- preamble DMAs. One tile per DMA (the tile framework
            # WAW-serializes multiple DMAs into one tile). Early set =
            # what the pre-HAM-grant trickle must deliver: x chunks + wr
            # (for the weight-free router pass-1 warm-up) + bi + wi. ----
            xh0_sb = wpool.tile([P, KH, SB], BF16, name="xh0")
            nc.sync.dma_start(out=xh0_sb, in_=xh0[:, :, :])
            wr_sb = wpool.tile([P, 2, KH, E], BF16, name="wr")
            nc.scalar.dma_start(out=wr_sb, in_=wr[:, :, :, :])
            bi_sb = wpool.tile([P, KF], F32, name="bi")
            nc.scalar.dma_start(out=bi_sb, in_=bi[:, :])
            wig = []
            for g in range(KF // WG):
                t = wpool.tile([P, KH, WG * P], BF16, name=f"wig{g}")
                nc.scalar.dma_start(out=t, in_=wi[:, g])
                wig.append(t)
            # deferred streams: triggers fire on the gpsimd queue inside
            # superblock 0's j-loop, gated by hT probes, so they are
            # just-in-time and cannot starve the early wi/x trickle
            xh1_sb = wpool.tile([P, KH, cap - SB], BF16, name="xh1")
            xl0_sb = wpool.tile([P, KH, SB], BF16, name="xl0")
            xl1_sb = wpool.tile([P, KH, cap - SB], BF16, name="xl1")
            wo_a = wpool.tile([P, KF // 2, H], BF16, name="wo_a")
            wo_b = wpool.tile([P, KF // 2, H], BF16, name="wo_b")
            # broadcasts (free-dim operands) via the software DGE
            bo_sb = wpool.tile([P, H], F32, name="bo")
            nc.gpsimd.dma_start(out=bo_sb, in_=bo[None, :].to_broadcast([P, H]))
            esel_sb = wpool.tile([P, E], F32, name="esel")
            nc.gpsimd.dma_start(out=esel_sb, in_=esel[None, :].to_broadcast([P, E]))
            # identity for the PE-mode transpose of the router logits
            id8 = wpool.tile([E, E], F32, name="id8")
            make_identity(nc, id8)

            # ---- warm burst: tiny matmuls gated only on the 25KB wr
            # DMA. Measured grant-neutral (too short to start the HAM
            # window) and ~0.7us of PE; kept because the measured-best
            # samples include it and it may help p-state ramp on entry ----
            warm = pr_pool.tile([E, TB], F32, tag="pr", name="warm")
            for w in range(24):
                nc.tensor.matmul(
                    warm[0:E, 0:E], lhsT=wr_sb[:, 0, w % KH, :],
                    rhs=wr_sb[:, 1, w % KH, :], start=True, stop=True,
                )

            SBv = SB

            def chunks_of(b):
                return [(c0, min(TB, b - c0)) for c0 in range(0, b, TB)]

            def rhs_xh(k, t0, b):
                if t0 < SBv:
                    return xh0_sb[:, k, t0 : t0 + b]
                return xh1_sb[:, k, t0 - SBv : t0 - SBv + b]

            def rhs_xl(k, t0, b):
                if t0 < SBv:
                    return xl0_sb[:, k, t0 : t0 + b]
                return xl1_sb[:, k, t0 - SBv : t0 - SBv + b]

            def emit_m1_chain(st, j, c0, cb):
                t0, hT = st["t0"], st["hT"]
                jj = j % WG
                ps = ph_pool.tile([P, cb], F32, tag="ph")
                for k in range(KH):
                    lhsT = wig[j // WG][:, k, jj * P : (jj + 1) * P]
                    nc.tensor.matmul(
                        ps, lhsT=lhsT, rhs=rhs_xh(k, t0 + c0, cb),
                        start=(k == 0), stop=(k == KH - 1),
                    )
                nc.scalar.activation(
                    out=hT[:, j, c0 : c0 + cb], in_=ps,
                    func=mybir.ActivationFunctionType.Gelu,
                    bias=bi_sb[:, j : j + 1], scale=1.0,
                )

            def emit_m1(st, j):
                for c0, cb in st["chunks"]:
                    emit_m1_chain(st, j, c0, cb)

            # router: logits^T = Wrh^T@xh (+ Wrh^T@xl + Wrl^T@xh later),
            # fp32 psum chained across the three passes per 512-chunk
            def emit_router_p1(st, c0, cb):
                t0, b = st["t0"], st["b"]
                if c0 == 0:
                    st["lgT"] = rpool.tile(
                        [E, b], F32, tag="lgT", name="lgT"
                    )
                    st["pr"] = {}
                pr = pr_pool.tile([E, cb], F32, tag="pr")
                st["pr"][c0] = pr
                for k in range(KH):
                    nc.tensor.matmul(
                        pr, lhsT=wr_sb[:, 0, k, :], rhs=rhs_xh(k, t0 + c0, cb),
                        start=(k == 0), stop=False,
                    )

            def emit_router_p23(st, c0, cb):
                t0, b = st["t0"], st["b"]
                pr = st["pr"][c0]
                for pi, (s, rf) in enumerate([(0, rhs_xl), (1, rhs_xh)]):
                    for k in range(KH):
                        nc.tensor.matmul(
                            pr, lhsT=wr_sb[:, s, k, :],
                            rhs=rf(k, t0 + c0, cb),
                            start=False, stop=(pi == 1 and k == KH - 1),
                        )
                nc.vector.tensor_copy(out=st["lgT"][:, c0 : c0 + cb], in_=pr)

            def emit_router_t(st):
                b, nts, lgT = st["b"], st["nts"], st["lgT"]
                prt = prt_pool.tile([P, nts, E], F32, tag="prt")
                lg = rpool.tile([P, nts, E], F32, tag="lg")
                for ts in range(nts):
                    tl_ = min(P, b - ts * P)
                    nc.tensor.transpose(
                        prt[0:tl_, ts, :], lgT[:, ts * P : ts * P + tl_], id8
                    )
                    nc.vector.tensor_copy(
                        out=lg[0:tl_, ts, :], in_=prt[0:tl_, ts, :]
                    )
                st["lg"] = lg

            def emit_router_v_pre(st, ts):
                # per-ts reductions into [P, nts] block temps (vector only)
                lg, nts = st["lg"], st["nts"]
                if ts == 0:
                    for nm in ("m1b", "m2b", "ddb", "lcb", "e2b", "w_blk"):
                        st[nm] = rpool.tile([P, nts], F32, tag=nm, name=nm)
                lg_s = lg[:, ts, :]
                m1 = st["m1b"][:, ts : ts + 1]
                nc.vector.reduce_max(m1, lg_s, axis=mybir.AxisListType.X)
                ge = rpool.tile([P, E], F32, tag="ge")
                nc.vector.tensor_scalar(
                    ge, lg_s, scalar1=m1, scalar2=-1e30,
                    op0=mybir.AluOpType.is_ge, op1=mybir.AluOpType.mult,
                )
                mk = rpool.tile([P, E], F32, tag="mk")
                nc.vector.tensor_tensor(mk, lg_s, ge, op=mybir.AluOpType.add)
                m2 = st["m2b"][:, ts : ts + 1]
                nc.vector.reduce_max(m2, mk, axis=mybir.AxisListType.X)
                nc.vector.tensor_tensor(
                    st["ddb"][:, ts : ts + 1], m2, m1,
                    op=mybir.AluOpType.subtract,
                )
                lc_t = rpool.tile([P, E], F32, tag="lct")
                nc.vector.tensor_tensor(
                    lc_t, lg_s, esel_sb, op=mybir.AluOpType.mult
                )
                nc.vector.reduce_sum(
                    st["lcb"][:, ts : ts + 1], lc_t, axis=mybir.AxisListType.X
                )

            def emit_router_v_exp(st):
                # ONE scalar-engine exp per superblock: avoids the ~1.5us
                # GELU<->EXP activation-table reload per ts that otherwise
                # stalls the mm1 gelu stream through the psum ring
                nc.scalar.activation(
                    st["e2b"], st["ddb"], mybir.ActivationFunctionType.Exp
                )

            def emit_router_v_post(st):
                # softmax-of-top2 -> weights for ALL ts at once (vector)
                nts = st["nts"]
                m1b, m2b, e2b, lcb = st["m1b"], st["m2b"], st["e2b"], st["lcb"]
                den = rpool.tile([P, nts], F32, tag="den", name="den")
                nc.vector.tensor_scalar_add(den, e2b, 1.0)
                w1 = rpool.tile([P, nts], F32, tag="w1", name="w1")
                nc.vector.reciprocal(w1, den)
                w2 = rpool.tile([P, nts], F32, tag="w2", name="w2")
                nc.vector.tensor_tensor(w2, e2b, w1, op=mybir.AluOpType.mult)
                d1 = rpool.tile([P, nts], F32, tag="d1", name="d1")
                nc.vector.tensor_tensor(d1, lcb, m1b, op=mybir.AluOpType.is_ge)
                g2 = rpool.tile([P, nts], F32, tag="g2", name="g2")
                nc.vector.tensor_tensor(g2, lcb, m2b, op=mybir.AluOpType.is_ge)
                wa = rpool.tile([P, nts], F32, tag="wa", name="wa")
                nc.vector.tensor_tensor(wa, w1, w2, op=mybir.AluOpType.subtract)
                t1 = rpool.tile([P, nts], F32, tag="t1", name="t1")
                nc.vector.tensor_tensor(t1, d1, wa, op=mybir.AluOpType.mult)
                t2 = rpool.tile([P, nts], F32, tag="t2", name="t2")
                nc.vector.tensor_tensor(t2, g2, w2, op=mybir.AluOpType.mult)
                nc.vector.tensor_tensor(
                    st["w_blk"], t1, t2, op=mybir.AluOpType.add
                )

            def emit_m2(st, ts):
                t0, b, hT, w_blk = st["t0"], st["b"], st["hT"], st["w_blk"]
                tl_ = min(P, b - ts * P)
                po_a = po_pool.tile([P, HO], F32, tag="po")
                po_b = po_pool.tile([P, HO], F32, tag="po")
                for j in range(KF):
                    lhsT = hT[:, j, ts * P : ts * P + tl_]
                    wot = wo_a if j < KF // 2 else wo_b
                    jw = j % (KF // 2)
                    nc.tensor.matmul(
                        po_a[0:tl_, :], lhsT=lhsT, rhs=wot[:, jw, 0:HO],
                        start=(j == 0), stop=(j == KF - 1),
                    )
                    nc.tensor.matmul(
                        po_b[0:tl_, :], lhsT=lhsT, rhs=wot[:, jw, HO : 2 * HO],
                        start=(j == 0), stop=(j == KF - 1),
                    )
                o = opool.tile([P, H], F32, tag="os")
                nc.vector.tensor_tensor(
                    o[0:tl_, 0:HO], po_a[0:tl_, :], bo_sb[0:tl_, 0:HO],
                    op=mybir.AluOpType.add,
                )
                nc.vector.tensor_tensor(
                    o[0:tl_, HO : 2 * HO], po_b[0:tl_, :],
                    bo_sb[0:tl_, HO : 2 * HO], op=mybir.AluOpType.add,
                )
                nc.vector.tensor_scalar_mul(
                    o[0:tl_, :], o[0:tl_, :], scalar1=w_blk[0:tl_, ts : ts + 1]
                )
                r0 = t0 + ts * P
                nc.sync.dma_start(out=out[r0 : r0 + tl_, :], in_=o[0:tl_, :])

            # ---- main loop: per superblock, j-major mm1, then router,
            # then mm2. For superblock 0 the router's x-only pass runs
            # FIRST (weight-free PE work that rides out the pre-HAM-grant
            # DMA trickle), and the deferred streams are triggered JIT off
            # gelu progress via gpsimd probes. ----
            deferred = {
                12: lambda: nc.gpsimd.dma_start(out=xh1_sb, in_=xh1[:, :, :]),
                16: lambda: nc.gpsimd.dma_start(out=xl0_sb, in_=xlo[:, :, 0:SB]),
                18: lambda: nc.gpsimd.dma_start(
                    out=xl1_sb, in_=xlo[:, :, SB:cap]
                ),
                20: lambda: nc.gpsimd.dma_start(
                    out=wo_a, in_=wo[:, 0 : KF // 2, :]
                ),
                22: lambda: nc.gpsimd.dma_start(
                    out=wo_b, in_=wo[:, KF // 2 : KF, :]
                ),
            }
            probe = wpool.tile([1, KF], F32, name="probe")
            first_sb = True
            for t0, b in blocks:
                nts = (b + P - 1) // P
                st = {
                    "t0": t0, "b": b, "nts": nts, "chunks": chunks_of(b),
                    "hT": hpool.tile([P, KF, b], BF16, tag="hT", name="hT"),
                }
                for j in range(KF):
                    emit_m1(st, j)
                    if first_sb and j in deferred:
                        nc.gpsimd.tensor_copy(
                            out=probe[:, j : j + 1],
                            in_=st["hT"][0:1, j, 0:1],
                        )
                        deferred[j]()
                first_sb = False
                for c0, cb in st["chunks"]:
                    emit_router_p1(st, c0, cb)
                    emit_router_p23(st, c0, cb)
                emit_router_t(st)
                for ts in range(nts):
                    emit_router_v_pre(st, ts)
                emit_router_v_exp(st)
                emit_router_v_post(st)
                for ts in range(nts):
                    emit_m2(st, ts)

    nc.compile()
    return nc


_NC_CACHE: dict = {}


def _get_nc(cap: int):
    if cap not in _NC_CACHE:
        _NC_CACHE[cap] = build_nc(cap)
    return _NC_CACHE[cap]


def _ensure_axon_hooks_module():
    """run_bass_kernel_spmd(trace=True) (e.g. via env BASS_TRACE=1) imports
    antenv.axon_hooks, which some images lack even though the boot code that
    would register the NTFF hook is present. Provide the module and register
    the real hook when available so tracing works instead of crashing."""
    try:
        import antenv.axon_hooks  # noqa: F401

        return
    except ImportError:
        pass
    try:
        import sys
        import types

        import antenv  # noqa: F401

        mod = types.ModuleType("antenv.axon_hooks")
        state = {"hook": None}
        mod.set_axon_ntff_profile_hook = lambda h: state.__setitem__("hook", h)
        mod.get_axon_ntff_profile_hook = lambda: state["hook"]
        try:
            from trn_agent_boot.trn_boot import _ntff_profile_via_ctypes

            mod.set_axon_ntff_profile_hook(
                _ntff_profile_via_ctypes("/opt/axon/libaxon_pjrt.so")
            )
        except Exception:
            pass
        sys.modules["antenv.axon_hooks"] = mod
    except Exception:
        pass


def _shard_tokens(xf, Wr):
    """Host-side dispatch: top-2 expert index per token (matches jax.lax.top_k
    tie-breaking: lowest index wins). Tokens whose 2nd/3rd logits are within
    1e-4 are also sent to the 3rd expert; the device routing chain gives the
    losing expert weight 0, so near-tie reorder on device is harmless."""
    logits = xf.astype(np.float32) @ np.asarray(Wr, np.float32).T  # [T, E]
    n = len(logits)
    ar = np.arange(n)
    i1 = np.argmax(logits, axis=1)
    l2 = logits.copy()
    l2[ar, i1] = -np.inf
    i2 = np.argmax(l2, axis=1)
    l3 = l2.copy()
    l3[ar, i2] = -np.inf
    i3 = np.argmax(l3, axis=1)
    tie = (l2[ar, i2] - l3[ar, i3]) < 1e-4
    tok_lists = []
    for c in range(N_CORES):
        tok_lists.append(
            np.concatenate([ar[i1 == c], ar[i2 == c], ar[tie & (i3 == c)]])
        )
    return tok_lists


def _pack_h(a):
    """[H_or_F rows, L] -> [P, rows//P, L] device layout (row r = k*P + p)."""
    r, L = a.shape
    return np.ascontiguousarray(a.reshape(r // P, P, L).transpose(1, 0, 2))


def kernel(x, Wr, Wi, bi, Wo, bo, _trace=False):
    x = np.asarray(x)
    xf = x.reshape(-1, H).astype(np.float32)
    tok_lists = _shard_tokens(xf, Wr)
    maxc = max(len(tl) for tl in tok_lists)
    cap = max(2 * TB + 64, int(np.ceil(maxc / 8) * 8))

    Wr32 = np.asarray(Wr, np.float32)
    WrT = Wr32.T  # [H, E]
    wr_hi = WrT.astype(BF16_NP)
    wr_lo = (WrT - wr_hi.astype(np.float32)).astype(BF16_NP)
    wr_dev = np.ascontiguousarray(
        np.stack([_pack_h(wr_hi), _pack_h(wr_lo)], axis=1)
    )  # [P, 2, KH, E] bf16

    in_maps = []
    for c in range(N_CORES):
        tl = tok_lists[c]
        xsel = xf[tl].T  # [H, L] fp32
        xh = np.zeros((H, cap), dtype=BF16_NP)
        xl = np.zeros((H, cap), dtype=BF16_NP)
        xh[:, : len(tl)] = xsel.astype(BF16_NP)
        xl[:, : len(tl)] = (xsel - xh[:, : len(tl)].astype(np.float32)).astype(
            BF16_NP
        )
        xh_dev = _pack_h(xh)  # [P, KH, cap]
        xl_dev = _pack_h(xl)
        wiT_dev = _pack_h(
            np.asarray(Wi[c], np.float32).T.astype(BF16_NP)
        )  # [P, KH, F]
        # grouped: [P, KF//WG, KH, WG*P], group g holds j-cols g*WG*P..(g+1)*WG*P
        wi_grp = np.ascontiguousarray(
            wiT_dev.reshape(P, KH, KF // WG, WG * P).transpose(0, 2, 1, 3)
        )
        woT_dev = _pack_h(
            np.asarray(Wo[c], np.float32).T.astype(BF16_NP)
        )  # [P, KF, H]
        sel = np.zeros(E, np.float32)
        sel[c] = 1.0
        in_maps.append(
            {
                "xh0": np.ascontiguousarray(xh_dev[:, :, : 2 * TB]),
                "xh1": np.ascontiguousarray(xh_dev[:, :, 2 * TB :]),
                "xlo": xl_dev,
                "wi": wi_grp,
                "wo": woT_dev,
                "wr": wr_dev,
                "bi": np.ascontiguousarray(
                    np.asarray(bi[c], np.float32).reshape(KF, P).T
                ),
                "bo": np.asarray(bo[c], np.float32),
                "esel": sel,
            }
        )

    _ensure_axon_hooks_module()
    nc = _get_nc(cap)
    res = run_bass_kernel_spmd(
        nc, in_maps, core_ids=list(range(N_CORES)), trace=_trace
    )

    # Unshard: scatter-add the per-expert (already routing-weighted) rows.
    out = np.zeros((T, H), dtype=np.float32)
    for c in range(N_CORES):
        tl = tok_lists[c]
        out[tl] += res.results[c]["out"][: len(tl)]
    out = out.reshape(x.shape)
    if _trace:
        return out, res
    return out
